# revision 18
# baseline (speedup 1.0000x reference)
"""Expert-parallel MoE GEGLU MLP (RMSNorm -> c_fc -> GEGLU -> c_proj) on 8
Trainium2 NeuronCores.

Sharding: expert-parallel. Core e computes the full MLP for expert e's tokens
(x[:, e] -> [8192, 768]); no collectives. gamma*sqrt(D) is folded into c_fc
and mult_bias into c_proj on the host, so the device kernel computes:

    u    = x @ W1g                      (bf16 x bf16 -> fp32 PSUM, raw x)
    s    = 1/||x||_2 per token          (DVE square+reduce, sqrt+recip)
    g    = gelu(u_gate*s) * (u_val*s)   (scale applied post-GEMM1: the norm
                                         is linear in the token axis, so it
                                         commutes with GEMM1 -- this takes
                                         normalization off the critical path)
    out  = g @ W2                       (bf16 x bf16 -> fp32 PSUM)

All inputs are re-laid-out on the host to partition-major tiles so every DMA
moves large contiguous per-partition blocks. W1 is loaded in m-chunk order so
the first GEMM1 chain can start ~11us in. The per-token scale is broadcast
across partitions with small bf16 selector matmuls. Norm work for superblock
sb+1 is injected mid-way through GEMM1(sb)'s chunk stream, and two GEMM1
chunks of sb+1 are emitted ahead of GEMM2(sb), so the PE never waits on the
GEGLU tail. GEGLU final multiplies run on GpSimd to keep the DVE FIFO free
for PSUM drains. Output DMAs alternate between the sync/scalar HWDGE queues.
"""

from contextlib import ExitStack

import ml_dtypes
import numpy as np

import concourse.bass as bass
import concourse.mybir as mybir
import concourse.tile as tile
from concourse import bacc
from concourse.bass_utils import run_bass_kernel_spmd
from concourse.masks import make_identity

# Problem dims (fixed by the nn_MLP_90795608637901 spec).
B, E, CAP, D = 8, 8, 1024, 768
H = 2048
H2 = 2 * H
T = B * CAP          # tokens per expert (per core) = 8192
SB = 1024            # tokens per super-block
NSB = T // SB        # 8
S = SB // 128        # 8 partition sub-tiles per super-block
KC1 = D // 128       # 6 contraction chunks for GEMM1
MC = H // 128        # 16 value/gate chunk pairs
KC2 = H // 128       # 16 contraction chunks for GEMM2
W1_FLAT = MC * 2 * KC1 * 128  # 24576 cols in the host-packed w1 layout

BF = mybir.dt.bfloat16
F32 = mybir.dt.float32
ALU = mybir.AluOpType
ACTF = mybir.ActivationFunctionType


def build_kernel(nsb: int = NSB,
                 # tensor_tensor_reduce faults the DVE on this runtime
                 # (engine exception); ACT Square+accum is the safe path.
                 use_ttr: bool = False,
                 use_sqrt_recip: bool = True,
                 use_gp_mul: bool = True,
                 use_bf16_sel: bool = True,
                 debug_stop: int = 0) -> bass.Bass:
    # debug_stop: 0=full, 1=startup DMAs only, 2=+norm, 3=+GEMM1 m0/m1 chains,
    # 4=+GEMM1 full incl GEGLU (no GEMM2)
    nc = bacc.Bacc("TRN2", target_bir_lowering=False, debug=False)

    # Host-packed layouts (see prepare_in_maps):
    #   xb[p, sb, s, d]   = x[sb*1024 + s*128 + p, d]   (token-major view)
    #   xt[p, sb, k, t]   = x[sb*1024 + t, k*128 + p]   (d-major view)
    #   w1[p, flat(m,t,k,c)] = W1g[k*128+p, t*2048 + m*128 + c]
    #   w2[p, k2, d]      = W2m[k2*128+p, d]
    xb = nc.declare_dram_parameter("xb", [128, NSB, S, D], BF, isOutput=False)
    xt = nc.declare_dram_parameter("xt", [128, NSB, KC1, SB], BF, isOutput=False)
    w1 = nc.declare_dram_parameter("w1", [128, W1_FLAT], BF, isOutput=False)
    w2 = nc.declare_dram_parameter("w2", [128, KC2, D], BF, isOutput=False)
    sel = nc.declare_dram_parameter("sel", [S, SB],
                                    BF if use_bf16_sel else F32,
                                    isOutput=False)
    out = nc.declare_dram_parameter("out", [T, D], BF, isOutput=True)

    with tile.TileContext(nc) as tc, ExitStack() as ctx:
        weights = ctx.enter_context(tc.tile_pool(name="weights", bufs=1))
        xtp = ctx.enter_context(tc.tile_pool(name="xtp", bufs=2))
        xbp = ctx.enter_context(tc.tile_pool(name="xbp", bufs=2))
        gpool = ctx.enter_context(tc.tile_pool(name="gpool", bufs=1))
        scp = ctx.enter_context(tc.tile_pool(name="scp", bufs=2))
        agp = ctx.enter_context(tc.tile_pool(name="agp", bufs=6))
        obp = ctx.enter_context(tc.tile_pool(name="obp", bufs=3))
        smallp = ctx.enter_context(tc.tile_pool(name="smallp", bufs=2))
        psum_mm = ctx.enter_context(tc.tile_pool(name="psum_mm", bufs=6, space="PSUM"))
        psum_bc = ctx.enter_context(tc.tile_pool(name="psum_bc", bufs=1, space="PSUM"))

        # ---- startup DMAs, queue-ordered for the critical path ----
        # scalar/sync HWDGE queues split xt0 (GEMM1's operand) then xb0 (norm
        # input); gpsimd SWDGE queue brings w1 in consumption order.
        xt_tiles = {}
        xb_tiles = {}

        def issue_x(sb):
            # single-queue full-tensor prefetch on the otherwise idle sync
            # engine: keeps ACT's instruction stream free of DMA issues
            # (a blocked DMA_DIRECT2D stalls every ACT op behind it).
            xbb = xbp.tile([128, S, D], BF, name="xbb", tag="xbb")
            nc.sync.dma_start(out=xbb, in_=xb[:, sb, :, :])
            xtb = xtp.tile([128, KC1, SB], BF, name="xtb", tag="xtb")
            nc.sync.dma_start(out=xtb, in_=xt[:, sb, :, :])
            xt_tiles[sb] = xtb
            xb_tiles[sb] = xbb

        # sb0: one fat-descriptor DMA per tensor (12KB/partition runs --
        # queue throughput is descriptor-rate-bound, so big runs matter).
        # xt0 rides the scalar queue (ACT's only DMA instruction, ahead of
        # the squares); xb0 leads the sync queue.
        xtb0 = xtp.tile([128, KC1, SB], BF, name="xtb", tag="xtb")
        xbb0 = xbp.tile([128, S, D], BF, name="xbb", tag="xbb")
        nc.sync.dma_start(out=xbb0, in_=xb[:, 0, :, :])
        nc.scalar.dma_start(out=xtb0, in_=xt[:, 0, :, :])
        xt_tiles[0] = xtb0
        xb_tiles[0] = xbb0
        # w1 packed flat per partition in (m, t, k, c) order; GEMM1 slices it
        # by index arithmetic. Loaded in 4 m-blocks so m=0 lands first.
        w1s = weights.tile([128, W1_FLAT], BF)

        def w1ap(m, t, k):
            base = (((m * 2) + t) * KC1 + k) * 128
            return w1s[:, base:base + 128]

        MCOLS = 2 * KC1 * 128  # flat cols per m-chunk
        mb_edges = [0, 2, 4, 8, MC]  # first blocks small so m=0 lands early
        for b0, b1 in zip(mb_edges[:-1], mb_edges[1:]):
            nc.gpsimd.dma_start(out=w1s[:, b0 * MCOLS:b1 * MCOLS],
                                in_=w1[:, b0 * MCOLS:b1 * MCOLS])
        sels = weights.tile([S, SB], BF if use_bf16_sel else F32)
        nc.gpsimd.dma_start(out=sels, in_=sel[:, :])
        w2s = weights.tile([128, KC2, D], BF)
        if nsb > 1:
            issue_x(1)
        nc.sync.dma_start(out=w2s, in_=w2[:, :, :])

        ident = weights.tile([128, 128], F32)
        make_identity(nc, ident)
        bias0 = weights.tile([128, 1], F32)
        nc.vector.memset(bias0, 0.0)
        biaseps = weights.tile([128, 1], F32)
        nc.vector.memset(biaseps, 1e-24)

        # ---- per-superblock norm scale: s = 1/||x_token|| ----
        # stage1 (DVE+ACT, no PE): squared norms token-major, sqrt, recip.
        # stage2 (PE+DVE): transpose to a row, broadcast across partitions.
        norm_st = {}
        scs = {}

        def norm_stage1(sb):
            xbb = xb_tiles.pop(sb)
            ssb = smallp.tile([128, S], F32, name="ssb")
            sq0 = smallp.tile([128, D], BF, name="sq0")
            sq1 = smallp.tile([128, D], BF, name="sq1")
            for s in range(S):
                if use_ttr:
                    nc.vector.tensor_tensor_reduce(
                        out=(sq0 if s % 2 == 0 else sq1), in0=xbb[:, s], in1=xbb[:, s],
                        scale=1.0, scalar=0.0, op0=ALU.mult, op1=ALU.add,
                        accum_out=ssb[:, s:s + 1],
                    )
                else:
                    nc.scalar.activation(
                        (sq0 if s % 2 == 0 else sq1), xbb[:, s], ACTF.Square,
                        bias=bias0, accum_out=ssb[:, s:s + 1],
                    )
            yb = smallp.tile([128, S], F32, name="yb")
            if use_sqrt_recip:
                rt = smallp.tile([128, S], F32, name="rt")
                nc.scalar.activation(rt, ssb, ACTF.Sqrt, bias=biaseps)
                nc.vector.reciprocal(yb, rt)
            else:
                I32 = mybir.dt.int32
                tb = smallp.tile([128, S], F32, name="tb")
                nc.vector.tensor_scalar(
                    out=yb.bitcast(I32), in0=ssb.bitcast(I32),
                    scalar1=1, scalar2=-1,
                    op0=ALU.logical_shift_right, op1=ALU.bitwise_xor,
                )
                nc.vector.tensor_scalar(
                    out=yb.bitcast(I32), in0=yb.bitcast(I32),
                    scalar1=0x5F375A60, scalar2=None, op0=ALU.add,
                )
                for _ in range(3):
                    nc.vector.tensor_mul(tb, yb, yb)
                    nc.vector.tensor_mul(tb, tb, ssb)
                    nc.vector.tensor_scalar(
                        out=tb, in0=tb, scalar1=-0.5, scalar2=1.5,
                        op0=ALU.mult, op1=ALU.add,
                    )
                    nc.vector.tensor_mul(yb, yb, tb)
            norm_st[sb] = yb
            # reserve the broadcast buffer now so GEMM1 chunks emitted before
            # stage2 can reference it; stage2 fills it.
            scs[sb] = scp.tile([128, SB], F32, name="sc", tag="sc")

        def norm_stage2(sb):
            yb = norm_st.pop(sb)
            yt = psum_bc.tile([S, 128], F32, name="yt", tag="bc", space="PSUM")
            nc.tensor.transpose(yt, yb, ident)
            yrow = smallp.tile([S, 128], BF if use_bf16_sel else F32,
                               name="yrow")
            nc.vector.tensor_copy(yrow, yt)
            sc = scs[sb]
            for half in range(2):
                psc = psum_bc.tile([128, 512], F32, name="psc", tag="bc",
                                   space="PSUM")
                for s in range(4):
                    sg = half * 4 + s
                    nc.tensor.matmul(
                        psc[:, s * 128:(s + 1) * 128],
                        lhsT=sels[:, sg * 128:(sg + 1) * 128],
                        rhs=yrow, start=True, stop=True,
                    )
                nc.vector.tensor_copy(sc[:, half * 512:(half + 1) * 512], psc)
            scs[sb] = sc

        # ---- GEMM1 chunk: one (m, h2) pair -> gbuf[:, m, cols] ----
        def g1_mm(sb, m, h2):
            xtb = xt_tiles[sb]
            cols = slice(h2 * 512, (h2 + 1) * 512)
            pg = psum_mm.tile([128, 512], F32, name="pg", tag="mm", space="PSUM")
            pv = psum_mm.tile([128, 512], F32, name="pv", tag="mm", space="PSUM")
            for k in range(KC1):
                nc.tensor.matmul(
                    pg, lhsT=w1ap(m, 1, k), rhs=xtb[:, k, cols],
                    start=(k == 0), stop=(k == KC1 - 1),
                )
            for k in range(KC1):
                nc.tensor.matmul(
                    pv, lhsT=w1ap(m, 0, k), rhs=xtb[:, k, cols],
                    start=(k == 0), stop=(k == KC1 - 1),
                )
            return pg, pv

        def g1_act(sb, m, h2, pg, pv):
            sc = scs[sb]
            gbuf = gbufs[sb]
            cols = slice(h2 * 512, (h2 + 1) * 512)
            spg = agp.tile([128, 512], F32, name="spg", tag="ag")
            nc.vector.tensor_mul(spg, pg, sc[:, cols])
            ag = agp.tile([128, 512], F32, name="ag", tag="ag")
            nc.scalar.activation(ag, spg, ACTF.Gelu, bias=bias0)
            spv = agp.tile([128, 512], F32, name="spv", tag="ag")
            nc.vector.tensor_mul(spv, pv, sc[:, cols])
            eng_mul = nc.gpsimd if use_gp_mul else nc.vector
            eng_mul.tensor_mul(gbuf[:, m, cols], spv, ag)

        def g1_chunk(sb, m, h2):
            pg, pv = g1_mm(sb, m, h2)
            g1_act(sb, m, h2, pg, pv)

        def gemm2(sb):
            gbuf = gbufs.pop(sb)
            assert nsb >= 1
            for mt in range(S):
                ob = obp.tile([128, D], BF, name="ob", tag="ob")
                for d0, d1 in ((0, 512), (512, 768)):
                    po = psum_mm.tile([128, d1 - d0], F32, name="po", tag="mm",
                                      space="PSUM")
                    for k2 in range(KC2):
                        nc.tensor.matmul(
                            po, lhsT=gbuf[:, k2, mt * 128:(mt + 1) * 128],
                            rhs=w2s[:, k2, d0:d1],
                            start=(k2 == 0), stop=(k2 == KC2 - 1),
                        )
                    nc.vector.tensor_copy(ob[:, d0:d1], po)
                # steady state: sync only (ACT must stay DMA-free, gpsimd's
                # FIFO is blocked by next-superblock gmuls -- an output DMA
                # queued behind them deadlocks the ob rotation against the
                # PSUM drain). Last superblock: no gmuls follow and ACT is
                # idle, so fan the tail across all three queues.
                if sb == nsb - 1:
                    eng = (nc.sync, nc.scalar, nc.gpsimd)[mt % 3]
                else:
                    eng = nc.sync
                eng.dma_start(
                    out=out[sb * SB + mt * 128:sb * SB + (mt + 1) * 128, :],
                    in_=ob,
                )

        # ---- main pipeline ----
        gbufs = {}
        stopped = False
        if debug_stop == 1:
            ob = obp.tile([128, D], BF, name="ob", tag="ob")
            nc.vector.tensor_copy(ob, xb_tiles[0][:, 0, :])
            nc.sync.dma_start(out=out[0:128, :], in_=ob)
            stopped = True
        if not stopped:
            norm_stage1(0)
        if not stopped:
            gbufs[0] = gpool.tile([128, KC2, SB], BF, name="gbuf", tag="gb")
            # sb0 head: emit m=0,1 chains before stage2 (PE starts as soon
            # as xt0+w1m0 land), but their GEGLU only after sc0 is written.
            pend0 = []
            if debug_stop == 2:
                norm_stage2(0)
                ob = obp.tile([128, D], BF, name="ob", tag="ob")
                nc.vector.tensor_copy(ob, scs[0][:, 0:D])
                nc.sync.dma_start(out=out[0:128, :], in_=ob)
                stopped = True
            if not stopped:
                for m in range(2):
                    for h2 in range(2):
                        pend0.append((m, h2) + g1_mm(0, m, h2))
                norm_stage2(0)
            if not stopped and debug_stop == 3:
                for m, h2, pg, pv in pend0:
                    ob = obp.tile([128, 512], BF, name="ob", tag="ob")
                    nc.vector.tensor_copy(ob, pg)
                    nc.vector.tensor_copy(ob, pv)
                    nc.sync.dma_start(out=out[m * 128:(m + 1) * 128, 0:512],
                                      in_=ob)
                stopped = True
            if not stopped:
                for m, h2, pg, pv in pend0:
                    g1_act(0, m, h2, pg, pv)
        for sb in range(nsb if not stopped else 0):
            for m in range(2, MC):
                if sb + 1 < nsb:
                    if m == 4:
                        norm_stage1(sb + 1)
                    if m == 10:
                        norm_stage2(sb + 1)
                g1_chunk(sb, m, 0)
                g1_chunk(sb, m, 1)
            if debug_stop == 4:
                ob = obp.tile([128, D], BF, name="ob", tag="ob")
                nc.vector.tensor_copy(ob, gbufs[0][:, 0, 0:D])
                nc.sync.dma_start(out=out[0:128, :], in_=ob)
                break
            if sb + 1 < nsb:
                gbufs[sb + 1] = gpool.tile([128, KC2, SB], BF, name="gbuf",
                                           tag="gb")
                for m in range(2):
                    g1_chunk(sb + 1, m, 0)
                    g1_chunk(sb + 1, m, 1)
            gemm2(sb)
            if sb + 2 < nsb:
                issue_x(sb + 2)

    nc.finalize()
    return nc


def prepare_in_maps(x, c_fc, c_proj, gamma, mult_bias, sel_bf16=True):
    bf16 = ml_dtypes.bfloat16
    g = gamma.astype(np.float32) * np.float32(np.sqrt(D))
    w1_all = (c_fc.astype(np.float32) * g[None, :, None]).astype(bf16)
    w2_all = (c_proj.astype(np.float32)
              * mult_bias.astype(np.float32)[None, :, None]).astype(bf16)
    xs = np.transpose(x, (1, 0, 2, 3)).reshape(E, T, D).astype(bf16)

    xbh = np.ascontiguousarray(
        xs.reshape(E, NSB, S, 128, D).transpose(0, 3, 1, 2, 4))
    xth = np.ascontiguousarray(
        xs.reshape(E, NSB, SB, KC1, 128).transpose(0, 4, 1, 3, 2))
    w1h = np.ascontiguousarray(
        w1_all.reshape(E, KC1, 128, 2, MC, 128).transpose(0, 2, 4, 3, 1, 5)
    ).reshape(E, 128, W1_FLAT)
    w2h = np.ascontiguousarray(
        w2_all.reshape(E, KC2, 128, D).transpose(0, 2, 1, 3))
    selh = np.zeros((S, SB), bf16 if sel_bf16 else np.float32)
    for s in range(S):
        selh[s, s * 128:(s + 1) * 128] = 1.0
    return [
        {"xb": xbh[e], "xt": xth[e], "w1": w1h[e], "w2": w2h[e], "sel": selh}
        for e in range(E)
    ]


def run(in_maps, trace: bool = False):
    nc = build_kernel()
    return run_bass_kernel_spmd(
        nc, in_maps, core_ids=list(range(E)), trace=trace,
    )


def kernel(x, c_fc, c_proj, gamma, mult_bias):
    in_maps = prepare_in_maps(x, c_fc, c_proj, gamma, mult_bias)
    res = run(in_maps)
    out = np.empty((E, B, CAP, D), np.float32)
    for e in range(E):
        out[e] = res.results[e]["out"].astype(np.float32).reshape(B, CAP, D)
    return np.ascontiguousarray(out.transpose(1, 0, 2, 3))
